# revision 8
# baseline (speedup 1.0000x reference)
"""Bayesian block-sparse linear layer (gnn message passing) on 8 Trainium2 cores.

out = segment_sum_e( v[e].T @ x_block[col_g[e]] ) + bias,
v[e] = eps_w[e] * exp(weight_log_var[e]) + weight_mean[e]   (32x32 blocks)

Strategy:
  * Batch-shard x across the 8 cores (128 columns each) -> one SPMD program.
  * The graph (row_g/col_g) is host-visible, so the program is specialized to
    it: every per-edge matmul is a 32x32 PE sub-array matmul (tile_position)
    so up to 16 edge-matmuls execute concurrently on the PE array.
  * All of x lives in SBUF in a block-permuted layout (block c at partition
    group c%4).  Edge weights are sampled on device (ACT exp + 2 DVE ops)
    from host-packed per-round layouts, then consumed as matmul lhsT.
  * Output blocks are processed in 16 rounds of 16 blocks; each block owns a
    [32, 512] PSUM strip = 4 private accumulator copies (one per PE row
    group) so concurrent sub-array matmuls never accumulate into the same
    PSUM element.  Evacuation sums the 4 copies and adds the sampled bias
    (ACT Identity with per-partition bias + 3 DVE adds).
"""

import os

import numpy as np

# problem dims (hardcoded per spec)
G1 = 256
G2 = 256
A1 = 32
A2 = 32
B = 1024
NCORES = 8
BSHARD = B // NCORES  # 128
NROUNDS = 16
BPR = 16    # blocks per round
NBANKS = 4  # PSUM banks per round
ZED = object()  # sentinel

LAST_PROFILE = None
_prog_cache = {}


def _dt_mode():
    return os.environ.get("BSL_DTYPE", "fp32")


# ---------------------------------------------------------------- host plan

def _plan(row_g, col_g):
    """Specialize the schedule to the graph."""
    E = len(row_g)
    blk = [[[] for _ in range(4)] for _ in range(G2)]
    for e in range(E):
        blk[int(row_g[e])][int(col_g[e]) % 4].append(e)
    cnts = np.array([[len(blk[q][g]) for g in range(4)] for q in range(G2)])

    # greedy bin-pack blocks into rounds, balancing per-partition-group load
    order = np.argsort(-cnts.sum(1), kind="stable")
    rounds = [[] for _ in range(NROUNDS)]
    load = np.zeros((NROUNDS, 4), np.int64)
    for q in order:
        best, bkey = None, None
        for r in range(NROUNDS):
            if len(rounds[r]) >= BPR:
                continue
            nl = load[r] + cnts[q]
            key = (int(nl.max()), int(nl.sum()))
            if best is None or key < bkey:
                best, bkey = r, key
        rounds[best].append(int(q))
        load[best] += cnts[q]

    blkmap = {}
    for r in range(NROUNDS):
        for idx, q in enumerate(rounds[r]):
            blkmap[q] = (r, idx)

    sched = []
    for r in range(NROUNDS):
        glists = [[] for _ in range(4)]
        for q in rounds[r]:
            for g in range(4):
                lst = blk[q][g]
                if lst:
                    for e in lst:
                        glists[g].append((e, q))
                else:
                    # region never written otherwise -> dummy zero matmul
                    glists[g].append((E, q))
        # round-robin by slot so consecutive same-group matmuls hit
        # different PE sub-arrays
        for g in range(4):
            byslot = [[], [], [], []]
            for e, q in glists[g]:
                byslot[blkmap[q][1] // 4].append((e, q))
            inter = []
            i = 0
            while any(byslot):
                sl = byslot[i % 4]
                if sl:
                    inter.append(sl.pop(0))
                i += 1
            glists[g] = inter
        L = max(len(x) for x in glists)
        q0 = rounds[r][0]
        for g in range(4):
            while len(glists[g]) < L:
                glists[g].append((E, q0))
        # start/stop flags per (row-group bank g, slot s): the first matmul
        # on tile (g, s) clears that bank's 2KB slot row, the last closes
        # the accumulation group.  Only tile (g, s) writes that region, so
        # the clear/accumulate ordering is the PE's own FIFO - race-free.
        first_pos, last_pos = {}, {}
        for p in range(L):
            for g in range(4):
                e, q = glists[g][p]
                s = blkmap[q][1] // 4
                if (g, s) not in first_pos:
                    first_pos[(g, s)] = p
                last_pos[(g, s)] = p
        entries = [[None] * L for _ in range(4)]
        widx = np.full((4, L), E, np.int64)
        for p in range(L):
            for g in range(4):
                e, q = glists[g][p]
                ridx = blkmap[q][1]
                s, u = ridx // 4, ridx % 4
                xcol = (int(col_g[e]) // 4) if e < E else 0
                entries[g][p] = (s, u, xcol,
                                 first_pos[(g, s)] == p,
                                 last_pos[(g, s)] == p)
                widx[g, p] = e
        sched.append({"L": L, "entries": entries, "widx": widx})
    return sched, rounds


# ---------------------------------------------------------------- host pack

def _pack_weights(w, sched, np_dt):
    w = np.asarray(w, np.float32).reshape(-1, A1, A2)
    wext = np.concatenate([w, np.zeros((1, A1, A2), np.float32)], 0)
    outs = []
    for sc in sched:
        t = wext[sc["widx"]]                       # [4, L, 32, 32]
        t = t.transpose(0, 2, 1, 3).reshape(128, 32 * sc["L"])
        outs.append(np.ascontiguousarray(t.astype(np_dt)))
    return outs


def _pack_x(xk, np_dt):  # xk [8192, BSHARD]
    t = xk.reshape(64, 4, 32, BSHARD).transpose(1, 2, 0, 3)
    return np.ascontiguousarray(t.reshape(128, 64 * BSHARD).astype(np_dt))


def _pack_bias(vec, rounds):  # vec [8192] fp32
    out = np.zeros((NROUNDS, NBANKS, 128), np.float32)
    for r in range(NROUNDS):
        for idx, q in enumerate(rounds[r]):
            s, bb = idx // 4, idx % 4
            out[r, bb, 32 * s:32 * s + 32] = vec[32 * q:32 * q + 32]
    return np.ascontiguousarray(
        out.transpose(2, 0, 1).reshape(128, NROUNDS * NBANKS))


def _unpack_out(op, rounds):  # op [NROUNDS, 128, 4*BSHARD] fp32
    t = op.reshape(NROUNDS, 4, 32, 4, BSHARD).transpose(0, 1, 3, 2, 4)
    res = np.zeros((G2, 32, BSHARD), np.float32)
    for r in range(NROUNDS):
        k = t[r].reshape(16, 32, BSHARD)
        for idx, q in enumerate(rounds[r]):
            res[q] = k[idx]
    return res.reshape(G2 * 32, BSHARD)


# ---------------------------------------------------------------- program

def _build(sched, dt_w):
    import concourse.bacc as bacc
    import concourse.mybir as mybir
    import concourse.tile as tile_mod

    nc = bacc.Bacc("TRN2", target_bir_lowering=False, debug=False,
                   num_devices=NCORES)
    f32 = mybir.dt.float32
    AF = mybir.ActivationFunctionType
    ADD = mybir.AluOpType.add
    MUL = mybir.AluOpType.mult

    x_d = nc.dram_tensor("x_packed", [128, 64 * BSHARD], dt_w,
                         kind="ExternalInput")
    wm_d = [nc.dram_tensor(f"wm_{r}", [128, 32 * sched[r]["L"]], dt_w,
                           kind="ExternalInput") for r in range(NROUNDS)]
    wl_d = [nc.dram_tensor(f"wl_{r}", [128, 32 * sched[r]["L"]], dt_w,
                           kind="ExternalInput") for r in range(NROUNDS)]
    we_d = [nc.dram_tensor(f"we_{r}", [128, 32 * sched[r]["L"]], dt_w,
                           kind="ExternalInput") for r in range(NROUNDS)]
    bm_d = nc.dram_tensor("bm_packed", [128, NROUNDS * NBANKS], f32,
                          kind="ExternalInput")
    bl_d = nc.dram_tensor("bl_packed", [128, NROUNDS * NBANKS], f32,
                          kind="ExternalInput")
    be_d = nc.dram_tensor("be_packed", [128, NROUNDS * NBANKS], f32,
                          kind="ExternalInput")
    out_d = nc.dram_tensor("out_packed", [NROUNDS, 128, NBANKS * BSHARD], f32,
                           kind="ExternalOutput")

    with tile_mod.TileContext(nc) as tc:
        with tc.tile_pool(name="xp", bufs=1) as xp, \
             tc.tile_pool(name="bp", bufs=1) as bp, \
             tc.tile_pool(name="wp", bufs=3) as wp, \
             tc.tile_pool(name="pp", bufs=8, space="PSUM") as pp, \
             tc.tile_pool(name="ep", bufs=8) as ep, \
             tc.tile_pool(name="opool", bufs=2) as opool:

            xt = xp.tile([128, 64 * BSHARD], dt_w, name="xt")
            nc.sync.dma_start(xt[:, :], x_d.ap())

            bmt = bp.tile([128, NROUNDS * NBANKS], f32, name="bmt")
            nc.sync.dma_start(bmt[:, :], bm_d.ap())
            blt = bp.tile([128, NROUNDS * NBANKS], f32, name="blt")
            nc.sync.dma_start(blt[:, :], bl_d.ap())
            bet = bp.tile([128, NROUNDS * NBANKS], f32, name="bet")
            nc.sync.dma_start(bet[:, :], be_d.ap())
            bias = bp.tile([128, NROUNDS * NBANKS], f32, name="bias")
            nc.scalar.activation(bias[:, :], blt[:, :], AF.Exp)
            nc.vector.tensor_tensor(bias[:, :], bias[:, :], bet[:, :], MUL)
            nc.vector.tensor_tensor(bias[:, :], bias[:, :], bmt[:, :], ADD)

            for r in range(NROUNDS):
                L = sched[r]["L"]
                W = 32 * L
                wlt = wp.tile([128, W], dt_w, tag="wl", name=f"wl_t{r}")
                nc.sync.dma_start(wlt[:, :], wl_d[r].ap())
                wet = wp.tile([128, W], dt_w, tag="we", name=f"we_t{r}")
                nc.sync.dma_start(wet[:, :], we_d[r].ap())
                wmt = wp.tile([128, W], dt_w, tag="wm", name=f"wm_t{r}")
                nc.sync.dma_start(wmt[:, :], wm_d[r].ap())

                # v = eps * exp(log_var) + mean, in column halves so the
                # first matmuls can start before the full round is sampled
                half = 32 * ((L + 1) // 2)
                for c0, c1 in ((0, half), (half, W)):
                    if c0 >= c1:
                        continue
                    nc.scalar.activation(wlt[:, c0:c1], wlt[:, c0:c1], AF.Exp)
                    nc.vector.tensor_tensor(wet[:, c0:c1], wet[:, c0:c1],
                                            wlt[:, c0:c1], MUL)
                    nc.vector.tensor_tensor(wet[:, c0:c1], wet[:, c0:c1],
                                            wmt[:, c0:c1], ADD)

                # bank g is private to PE row-group g: no two sub-array
                # matmuls ever write the same (bank, partition-range).
                banks = [pp.tile([128, NBANKS * BSHARD], f32, tag="bank",
                                 name=f"bank{r}_{b}") for b in range(4)]
                ents = sched[r]["entries"]
                for p in range(L):
                    for g in range(4):
                        s, u, xcol, st, sp = ents[g][p]
                        nc.tensor.matmul(
                            banks[g][32 * s:32 * s + 32,
                                     BSHARD * u:BSHARD * u + BSHARD],
                            lhsT=wet[32 * g:32 * g + 32, 32 * p:32 * p + 32],
                            rhs=xt[32 * g:32 * g + 32,
                                   BSHARD * xcol:BSHARD * xcol + BSHARD],
                            start=st, stop=sp, skip_group_check=True,
                            tile_position=(32 * g, 32 * s))

                # out = sum of the 4 row-group copies + bias
                # (DVE can take at most one PSUM operand per instruction)
                t1 = ep.tile([128, NBANKS * BSHARD], f32, tag="eacc",
                             name=f"t1_{r}")
                nc.scalar.activation(t1[:, :], banks[0][:, :], AF.Identity,
                                     bias=0.0)
                nc.vector.tensor_tensor(t1[:, :], t1[:, :], banks[1][:, :],
                                        ADD)
                nc.vector.tensor_tensor(t1[:, :], t1[:, :], banks[2][:, :],
                                        ADD)
                nc.vector.tensor_tensor(t1[:, :], t1[:, :], banks[3][:, :],
                                        ADD)
                ot = opool.tile([128, NBANKS * BSHARD], f32, tag="ot",
                                name=f"ot{r}")
                for u in range(4):
                    nc.scalar.activation(
                        ot[:, BSHARD * u:BSHARD * (u + 1)],
                        t1[:, BSHARD * u:BSHARD * (u + 1)], AF.Identity,
                        bias=bias[:, 4 * r + u:4 * r + u + 1])
                nc.sync.dma_start(out_d.ap()[r, :, :], ot[:, :])

    nc.compile()
    return nc


# ---------------------------------------------------------------- entry

def _get_program(row_g, col_g):
    import concourse.mybir as mybir
    import ml_dtypes
    mode = _dt_mode()
    key = (row_g.tobytes(), col_g.tobytes(), mode)
    if key not in _prog_cache:
        sched, rounds = _plan(row_g, col_g)
        dt_w = mybir.dt.float32 if mode == "fp32" else mybir.dt.bfloat16
        np_dt = np.float32 if mode == "fp32" else ml_dtypes.bfloat16
        nc = _build(sched, dt_w)
        _prog_cache[key] = (nc, sched, rounds, np_dt)
    return _prog_cache[key]


def make_in_maps(inputs):
    """Host-side shard + pack.  Returns (nc, in_maps, rounds)."""
    row_g = np.asarray(inputs["row_g"])
    col_g = np.asarray(inputs["col_g"])
    nc, sched, rounds, np_dt = _get_program(row_g, col_g)

    x = np.asarray(inputs["x"], np.float32)
    wm = _pack_weights(inputs["weight_mean"], sched, np_dt)
    wl = _pack_weights(inputs["weight_log_var"], sched, np_dt)
    we = _pack_weights(inputs["eps_w"], sched, np_dt)
    bm = _pack_bias(np.asarray(inputs["b_mean"], np.float32), rounds)
    bl = _pack_bias(np.asarray(inputs["b_log_var"], np.float32), rounds)
    be = _pack_bias(np.asarray(inputs["eps_b"], np.float32), rounds)

    shared = {}
    for r in range(NROUNDS):
        shared[f"wm_{r}"] = wm[r]
        shared[f"wl_{r}"] = wl[r]
        shared[f"we_{r}"] = we[r]
    shared["bm_packed"] = bm
    shared["bl_packed"] = bl
    shared["be_packed"] = be

    in_maps = []
    for k in range(NCORES):
        m = dict(shared)
        m["x_packed"] = _pack_x(
            np.ascontiguousarray(x[:, k * BSHARD:(k + 1) * BSHARD]), np_dt)
        in_maps.append(m)
    return nc, in_maps, rounds


def kernel(**inputs):
    global LAST_PROFILE
    from concourse import bass_utils

    nc, in_maps, rounds = make_in_maps(inputs)
    trace = os.environ.get("BSL_TRACE", "0") == "1"
    res = bass_utils.run_bass_kernel_spmd(
        nc, in_maps, core_ids=list(range(NCORES)), trace=trace)
    LAST_PROFILE = {
        "exec_time_ns": res.exec_time_ns,
        "mean_exec_time_ns": res.mean_exec_time_ns,
        "max_exec_time_core_id": res.max_exec_time_core_id,
        "trace": (res.instructions_and_trace[1]
                  if res.instructions_and_trace else None),
        "insts": (res.instructions_and_trace[0]
                  if res.instructions_and_trace else None),
    }
    out = np.zeros((G2 * A2, B), np.float32)
    for k in range(NCORES):
        out[:, k * BSHARD:(k + 1) * BSHARD] = _unpack_out(
            res.results[k]["out_packed"], rounds)
    return out, np.float32(0.0)


# revision 13
# speedup vs baseline: 1.7746x; 1.7746x over previous
"""Bayesian block-sparse linear layer (gnn message passing) on 8 Trainium2 cores.

out = segment_sum_e( v[e].T @ x_block[col_g[e]] ) + bias,
v[e] = eps_w[e] * exp(weight_log_var[e]) + weight_mean[e]   (32x32 blocks)

Measured on TRN2: the PE is instruction-issue bound at ~33ns per
(LDWEIGHTS+MATMUL) pair regardless of the moving free dim N, and 32x32
sub-array matmuls (tile_position) run concurrently as long as no two
sub-arrays write the same (PSUM bank, partition range).  So the design
maximizes work per matmul instruction: N=512 batch columns per matmul.

Sharding: output blocks are split across 4 distinct specialized programs
(one per row-group of the graph), each run on 2 cores that hold the two
512-column batch halves.  Within a program:
  * all of x (one batch half) lives in SBUF, block c at partition group
    c%4, columns 512*(c//4);
  * per-edge weights are sampled on device (ACT exp + 2 DVE ops) from
    host-packed per-round layouts and consumed as matmul lhsT;
  * rounds of 8 output blocks; block (gg, s) accumulates in PSUM bank
    4*gg+g for PE row-group g (4 private copies -> concurrent sub-array
    matmuls never touch the same bank+partitions, which hard-crashes);
  * evacuation sums the 4 copies and adds the sampled bias.
"""

import os

import numpy as np

# problem dims (hardcoded per spec)
G1 = 256
G2 = 256
A1 = 32
A2 = 32
B = 1024
NCORES = 8

NPROG = 4
CPP = 2            # cores per program (batch halves)
NW = B // CPP      # 512 batch columns per core
NROUNDS = 8        # per program
BPR = 8            # blocks per round
BPP = G2 // NPROG  # blocks per program

LAST_PROFILE = None
_prog_cache = {}


def _dt_mode():
    return os.environ.get("BSL_DTYPE", "bf16")


# ---------------------------------------------------------------- host plan

def _plan(row_g, col_g):
    """Specialize schedules to the graph: 4 programs x 8 rounds x 8 blocks."""
    E = len(row_g)
    blk = [[[] for _ in range(4)] for _ in range(G2)]
    for e in range(E):
        blk[int(row_g[e])][int(col_g[e]) % 4].append(e)
    cnts = np.array([[len(blk[q][g]) for g in range(4)] for q in range(G2)])

    # blocks -> programs, balancing total edge count
    order = np.argsort(-cnts.sum(1), kind="stable")
    progs = [[] for _ in range(NPROG)]
    ptot = np.zeros(NPROG, np.int64)
    for q in order:
        cand = [p for p in range(NPROG) if len(progs[p]) < BPP]
        p = min(cand, key=lambda p: ptot[p])
        progs[p].append(int(q))
        ptot[p] += cnts[q].sum()

    plans = []
    for p in range(NPROG):
        # blocks -> rounds, balancing per-partition-group load
        rounds = [[] for _ in range(NROUNDS)]
        load = np.zeros((NROUNDS, 4), np.int64)
        for q in sorted(progs[p], key=lambda q: -cnts[q].sum()):
            best, bkey = None, None
            for r in range(NROUNDS):
                if len(rounds[r]) >= BPR:
                    continue
                nl = load[r] + cnts[q]
                key = (int(nl.max()), int(nl.sum()))
                if best is None or key < bkey:
                    best, bkey = r, key
            rounds[best].append(q)
            load[best] += cnts[q]

        blkmap = {}
        for r in range(NROUNDS):
            for idx, q in enumerate(rounds[r]):
                blkmap[q] = (r, idx)

        sched = []
        for r in range(NROUNDS):
            glists = [[] for _ in range(4)]
            for q in rounds[r]:
                for g in range(4):
                    lst = blk[q][g]
                    if lst:
                        for e in lst:
                            glists[g].append((e, q))
                    else:
                        # region never written otherwise -> dummy zero mm
                        glists[g].append((E, q))
            # round-robin by slot so consecutive same-group matmuls hit
            # different PE sub-arrays
            for g in range(4):
                byslot = [[], [], [], []]
                for e, q in glists[g]:
                    byslot[blkmap[q][1] % 4].append((e, q))
                inter = []
                i = 0
                while any(byslot):
                    sl = byslot[i % 4]
                    if sl:
                        inter.append(sl.pop(0))
                    i += 1
                glists[g] = inter
            L = max(len(x) for x in glists)
            q0 = rounds[r][0]
            for g in range(4):
                while len(glists[g]) < L:
                    glists[g].append((E, q0))
            # start/stop per (row-group g, block q): each (bank, slot)
            # region belongs to exactly one block copy, and only PE tile
            # (g, s) writes it -> the clear/accumulate order is the PE's
            # own FIFO, race-free.
            first_pos, last_pos = {}, {}
            for pp in range(L):
                for g in range(4):
                    e, q = glists[g][pp]
                    if (g, q) not in first_pos:
                        first_pos[(g, q)] = pp
                    last_pos[(g, q)] = pp
            entries = [[None] * L for _ in range(4)]
            widx = np.full((4, L), E, np.int64)
            for pp in range(L):
                for g in range(4):
                    e, q = glists[g][pp]
                    ridx = blkmap[q][1]
                    gg, s = ridx // 4, ridx % 4
                    xcol = (int(col_g[e]) // 4) if e < E else 0
                    entries[g][pp] = (gg, s, xcol,
                                      first_pos[(g, q)] == pp,
                                      last_pos[(g, q)] == pp)
                    widx[g, pp] = e
            sched.append({"L": L, "entries": entries, "widx": widx})
        plans.append({"sched": sched, "rounds": rounds})
    return plans


# ---------------------------------------------------------------- host pack

def _pack_weights(w, sched, np_dt):
    w = np.asarray(w, np.float32).reshape(-1, A1, A2)
    wext = np.concatenate([w, np.zeros((1, A1, A2), np.float32)], 0)
    outs = []
    for sc in sched:
        t = wext[sc["widx"]]                       # [4, L, 32, 32]
        t = t.transpose(0, 2, 1, 3).reshape(128, 32 * sc["L"])
        outs.append(np.ascontiguousarray(t.astype(np_dt)))
    return outs


def _pack_x(xk, np_dt):  # xk [8192, NW]
    t = xk.reshape(64, 4, 32, NW).transpose(1, 2, 0, 3)
    return np.ascontiguousarray(t.reshape(128, 64 * NW).astype(np_dt))


def _pack_bias(vec, rounds):  # vec [8192] fp32 -> [128, NROUNDS*2]
    out = np.zeros((NROUNDS, 2, 128), np.float32)
    for r in range(NROUNDS):
        for idx, q in enumerate(rounds[r]):
            gg, s = idx // 4, idx % 4
            out[r, gg, 32 * s:32 * s + 32] = vec[32 * q:32 * q + 32]
    return np.ascontiguousarray(out.transpose(2, 0, 1).reshape(128, NROUNDS * 2))


def _unpack_out(op, rounds):  # op [NROUNDS, 128, 2*NW] fp32 -> [BPP*32, NW]
    res = np.zeros((G2, 32, NW), np.float32)
    t = op.reshape(NROUNDS, 4, 32, 2, NW).transpose(0, 3, 1, 2, 4)
    for r in range(NROUNDS):
        k = t[r].reshape(8, 32, NW)          # idx = 4*gg + s
        for idx, q in enumerate(rounds[r]):
            res[q] = k[idx]
    return res


# ---------------------------------------------------------------- program

def _build(sched, dt_w, pidx):
    import concourse.bacc as bacc
    import concourse.mybir as mybir
    import concourse.tile as tile_mod

    nc = bacc.Bacc("TRN2", target_bir_lowering=False, debug=False,
                   num_devices=CPP)
    f32 = mybir.dt.float32
    AF = mybir.ActivationFunctionType
    ADD = mybir.AluOpType.add
    MUL = mybir.AluOpType.mult

    x_d = nc.dram_tensor("x_packed", [128, 64 * NW], dt_w,
                         kind="ExternalInput")
    wm_d = [nc.dram_tensor(f"wm_{r}", [128, 32 * sched[r]["L"]], dt_w,
                           kind="ExternalInput") for r in range(NROUNDS)]
    wl_d = [nc.dram_tensor(f"wl_{r}", [128, 32 * sched[r]["L"]], dt_w,
                           kind="ExternalInput") for r in range(NROUNDS)]
    we_d = [nc.dram_tensor(f"we_{r}", [128, 32 * sched[r]["L"]], dt_w,
                           kind="ExternalInput") for r in range(NROUNDS)]
    bm_d = nc.dram_tensor("bm_packed", [128, NROUNDS * 2], f32,
                          kind="ExternalInput")
    bl_d = nc.dram_tensor("bl_packed", [128, NROUNDS * 2], f32,
                          kind="ExternalInput")
    be_d = nc.dram_tensor("be_packed", [128, NROUNDS * 2], f32,
                          kind="ExternalInput")
    out_d = nc.dram_tensor("out_packed", [NROUNDS, 128, 2 * NW], f32,
                           kind="ExternalOutput")

    with tile_mod.TileContext(nc) as tc:
        with tc.tile_pool(name="xp", bufs=1) as xp, \
             tc.tile_pool(name="bp", bufs=1) as bp, \
             tc.tile_pool(name="wp", bufs=3) as wp, \
             tc.tile_pool(name="pp", bufs=8, space="PSUM") as pp, \
             tc.tile_pool(name="ep", bufs=4) as ep, \
             tc.tile_pool(name="opool", bufs=2) as opool:

            xt = xp.tile([128, 64 * NW], dt_w, name="xt")
            nc.sync.dma_start(xt[:, :], x_d.ap())

            bmt = bp.tile([128, NROUNDS * 2], f32, name="bmt")
            nc.sync.dma_start(bmt[:, :], bm_d.ap())
            blt = bp.tile([128, NROUNDS * 2], f32, name="blt")
            nc.sync.dma_start(blt[:, :], bl_d.ap())
            bet = bp.tile([128, NROUNDS * 2], f32, name="bet")
            nc.sync.dma_start(bet[:, :], be_d.ap())
            bias = bp.tile([128, NROUNDS * 2], f32, name="bias")
            nc.scalar.activation(bias[:, :], blt[:, :], AF.Exp)
            nc.vector.tensor_tensor(bias[:, :], bias[:, :], bet[:, :], MUL)
            nc.vector.tensor_tensor(bias[:, :], bias[:, :], bmt[:, :], ADD)

            for r in range(NROUNDS):
                L = sched[r]["L"]
                W = 32 * L
                wlt = wp.tile([128, W], dt_w, tag="wl", name=f"wl_t{r}")
                nc.sync.dma_start(wlt[:, :], wl_d[r].ap())
                wet = wp.tile([128, W], dt_w, tag="we", name=f"we_t{r}")
                nc.sync.dma_start(wet[:, :], we_d[r].ap())
                wmt = wp.tile([128, W], dt_w, tag="wm", name=f"wm_t{r}")
                nc.sync.dma_start(wmt[:, :], wm_d[r].ap())

                # v = eps * exp(log_var) + mean, in column halves so the
                # first matmuls can start before the full round is sampled
                half = 32 * ((L + 1) // 2)
                for c0, c1 in ((0, half), (half, W)):
                    if c0 >= c1:
                        continue
                    nc.scalar.activation(wlt[:, c0:c1], wlt[:, c0:c1], AF.Exp)
                    nc.vector.tensor_tensor(wet[:, c0:c1], wet[:, c0:c1],
                                            wlt[:, c0:c1], MUL)
                    nc.vector.tensor_tensor(wet[:, c0:c1], wet[:, c0:c1],
                                            wmt[:, c0:c1], ADD)

                # bank 4*gg+g is private to PE row-group g: no two
                # sub-array matmuls ever write the same (bank, partitions)
                banks = [pp.tile([128, NW], f32, tag="bank",
                                 name=f"bank{r}_{b}") for b in range(8)]
                ents = sched[r]["entries"]
                for pp_i in range(L):
                    for g in range(4):
                        gg, s, xcol, st, sp = ents[g][pp_i]
                        nc.tensor.matmul(
                            banks[4 * gg + g][32 * s:32 * s + 32, 0:NW],
                            lhsT=wet[32 * g:32 * g + 32,
                                     32 * pp_i:32 * pp_i + 32],
                            rhs=xt[32 * g:32 * g + 32,
                                   NW * xcol:NW * xcol + NW],
                            start=st, stop=sp, skip_group_check=True,
                            tile_position=(32 * g, 32 * s))

                # out = sum of the 4 row-group copies + bias
                # (DVE can take at most one PSUM operand per instruction)
                ot = opool.tile([128, 2 * NW], f32, tag="ot", name=f"ot{r}")
                for gg in range(2):
                    t1 = ep.tile([128, NW], f32, tag="eacc",
                                 name=f"t1_{r}_{gg}")
                    nc.scalar.activation(t1[:, :], banks[4 * gg][:, :],
                                         AF.Identity, bias=0.0)
                    for g in range(1, 4):
                        nc.vector.tensor_tensor(
                            t1[:, :], t1[:, :], banks[4 * gg + g][:, :], ADD)
                    nc.scalar.activation(
                        ot[:, NW * gg:NW * (gg + 1)], t1[:, :], AF.Identity,
                        bias=bias[:, 2 * r + gg:2 * r + gg + 1])
                nc.sync.dma_start(out_d.ap()[r, :, :], ot[:, :])

    nc.compile()
    return nc


# ---------------------------------------------------------------- runner

def _dispatch_pjrt(nc, in_maps, devices):
    """Async-dispatch one Bass program on a specific device subset.

    Clone of concourse.bass2jax.run_bass_via_pjrt's multi-core branch with
    an explicit device list; returns un-materialized jax arrays so several
    programs can execute concurrently on disjoint core pairs.
    """
    import jax
    import concourse.mybir as mybir
    from concourse import bass2jax
    from jax.sharding import Mesh, PartitionSpec
    from jax.experimental.shard_map import shard_map

    bass2jax.install_neuronx_cc_hook()
    assert nc.dbg_addr is None
    partition_name = (nc.partition_id_tensor.name
                      if nc.partition_id_tensor else None)

    in_names, out_names, out_avals, zero_outs = [], [], [], []
    for alloc in nc.m.functions[0].allocations:
        if not isinstance(alloc, mybir.MemoryLocationSet):
            continue
        name = alloc.memorylocations[0].name
        if alloc.kind == "ExternalInput":
            if name != partition_name:
                in_names.append(name)
        elif alloc.kind == "ExternalOutput":
            out_names.append(name)
            shape = tuple(alloc.tensor_shape)
            dtype = mybir.dt.np(alloc.dtype)
            out_avals.append(jax.core.ShapedArray(shape, dtype))
            zero_outs.append(np.zeros(shape, dtype))
    n_params = len(in_names)
    n_outs = len(out_avals)
    in_names = in_names + out_names
    if partition_name is not None:
        in_names.append(partition_name)
    donate = tuple(range(n_params, n_params + n_outs))

    def _body(*args):
        operands = list(args)
        if partition_name is not None:
            operands.append(bass2jax.partition_id_tensor())
        outs = bass2jax._bass_exec_p.bind(
            *operands,
            out_avals=tuple(out_avals),
            in_names=tuple(in_names),
            out_names=tuple(out_names),
            lowering_input_output_aliases=(),
            sim_require_finite=True,
            sim_require_nnan=True,
            nc=nc,
        )
        return tuple(outs)

    n_cores = len(devices)
    mesh = Mesh(np.asarray(devices), ("core",))
    sharded = jax.jit(
        shard_map(_body, mesh=mesh,
                  in_specs=(PartitionSpec("core"),) * (n_params + n_outs),
                  out_specs=(PartitionSpec("core"),) * n_outs,
                  check_rep=False),
        donate_argnums=donate, keep_unused=True)
    per_core = [[np.asarray(m[name]) for name in in_names[:n_params]]
                for m in in_maps]
    concat_in = [np.concatenate([per_core[c][i] for c in range(n_cores)], 0)
                 for i in range(n_params)]
    concat_zeros = [np.zeros((n_cores * z.shape[0], *z.shape[1:]), z.dtype)
                    for z in zero_outs]
    out_arrs = sharded(*concat_in, *concat_zeros)
    return out_arrs, out_names, out_avals, n_cores


def _run_programs(ncs, maps_list, trace):
    """Run the programs concurrently on disjoint device pairs."""
    import jax
    devices = jax.devices()

    def dispatch_all():
        handles = []
        for q, (nc, maps) in enumerate(zip(ncs, maps_list)):
            devs = devices[CPP * q:CPP * (q + 1)]
            handles.append(_dispatch_pjrt(nc, maps, devs))
        return handles

    prof = {"exec_time_ns": None, "mean_exec_time_ns": None,
            "max_exec_time_core_id": None, "trace": None, "insts": None,
            "per_core_ns": None}

    if not trace:
        handles = dispatch_all()
    else:
        handles = None
        try:
            import glob as globmod
            import re
            import shutil
            import tempfile
            import time as time_mod
            from antenv.axon_hooks import get_axon_ntff_profile_hook
            hook = get_axon_ntff_profile_hook()
            neff_dir = tempfile.mkdtemp()
            with hook(neff_dir, list(range(NCORES))):
                t0 = time_mod.time()
                handles = dispatch_all()
                for out_arrs, _, _, _ in handles:
                    for a in out_arrs:
                        a.block_until_ready()
                wall_s = time_mod.time() - t0
            prof["wall_s"] = wall_s
            ntffs = globmod.glob(os.path.join(neff_dir, "*_body*.ntff"))
            if ntffs:
                import gauge.profiler
                from concourse._compat import FishPath
                # group by executable (one per program, in dispatch order)
                exids = sorted({re.search(r"executable(\d+)", f).group(1)
                                for f in ntffs})
                per_core = {}
                insts = None
                tracep = None
                best = -1
                for qi, exid in enumerate(exids):
                    sub = os.path.join(neff_dir, f"ex{exid}")
                    os.makedirs(sub, exist_ok=True)
                    for f in os.listdir(neff_dir):
                        if f"executable{exid}" in f:
                            shutil.move(os.path.join(neff_dir, f),
                                        os.path.join(sub, f))
                    profile = gauge.profiler.Profile(
                        profile_path=FishPath(sub),
                        kernel_dev_mode=True, profile_on_exit=False,
                        bass_kernel=ncs[min(qi, len(ncs) - 1)].m,
                        offline_processing=True, fname="*_body*")
                    results = profile.to_perfetto(model_index=(0, 1))
                    for ci, pr in enumerate(results or []):
                        per_core[(qi, ci)] = pr.exec_time_ns
                        if (pr.exec_time_ns or 0) > best:
                            best = pr.exec_time_ns or 0
                            insts, tracep = pr.insts, pr.trace_path
                vals = [v for v in per_core.values() if v]
                if vals:
                    prof.update(
                        exec_time_ns=max(vals),
                        mean_exec_time_ns=sum(vals) / len(vals),
                        max_exec_time_core_id=str(max(
                            per_core, key=lambda c: per_core[c] or 0)),
                        per_core_ns={str(k): v for k, v in per_core.items()},
                        insts=insts, trace=tracep)
        except Exception as exc:  # profiling must never break the run
            print(f"[kernel] trace failed: {type(exc).__name__}: {exc}")
            if handles is None:
                handles = dispatch_all()

    results = []
    for out_arrs, out_names, out_avals, n_cores in handles:
        cores = []
        for c in range(n_cores):
            cores.append({
                name: np.asarray(out_arrs[i]).reshape(
                    n_cores, *out_avals[i].shape)[c]
                for i, name in enumerate(out_names)})
        results.append(cores)
    return results, prof


# ---------------------------------------------------------------- entry

def _get_programs(row_g, col_g):
    import concourse.mybir as mybir
    import ml_dtypes
    mode = _dt_mode()
    key = (row_g.tobytes(), col_g.tobytes(), mode)
    if key not in _prog_cache:
        plans = _plan(row_g, col_g)
        dt_w = mybir.dt.float32 if mode == "fp32" else mybir.dt.bfloat16
        np_dt = np.float32 if mode == "fp32" else ml_dtypes.bfloat16
        ncs = [_build(plans[p]["sched"], dt_w, p) for p in range(NPROG)]
        _prog_cache[key] = (ncs, plans, np_dt)
    return _prog_cache[key]


def kernel(**inputs):
    global LAST_PROFILE

    row_g = np.asarray(inputs["row_g"])
    col_g = np.asarray(inputs["col_g"])
    ncs, plans, np_dt = _get_programs(row_g, col_g)

    x = np.asarray(inputs["x"], np.float32)
    xpk = [_pack_x(np.ascontiguousarray(x[:, h * NW:(h + 1) * NW]), np_dt)
           for h in range(CPP)]
    bm = np.asarray(inputs["b_mean"], np.float32)
    bl = np.asarray(inputs["b_log_var"], np.float32)
    be = np.asarray(inputs["eps_b"], np.float32)

    maps_list = []
    for p in range(NPROG):
        sched, rounds = plans[p]["sched"], plans[p]["rounds"]
        wm = _pack_weights(inputs["weight_mean"], sched, np_dt)
        wl = _pack_weights(inputs["weight_log_var"], sched, np_dt)
        we = _pack_weights(inputs["eps_w"], sched, np_dt)
        shared = {f"wm_{r}": wm[r] for r in range(NROUNDS)}
        shared.update({f"wl_{r}": wl[r] for r in range(NROUNDS)})
        shared.update({f"we_{r}": we[r] for r in range(NROUNDS)})
        shared["bm_packed"] = _pack_bias(bm, rounds)
        shared["bl_packed"] = _pack_bias(bl, rounds)
        shared["be_packed"] = _pack_bias(be, rounds)
        maps_list.append([{**shared, "x_packed": xpk[h]} for h in range(CPP)])

    trace = os.environ.get("BSL_TRACE", "0") == "1"
    results, prof = _run_programs(ncs, maps_list, trace)
    LAST_PROFILE = prof

    out = np.zeros((G2 * A2, B), np.float32)
    for p in range(NPROG):
        rounds = plans[p]["rounds"]
        mask = np.zeros(G2, bool)
        for r in range(NROUNDS):
            for q in rounds[r]:
                mask[q] = True
        rows = np.repeat(mask, 32)
        for h in range(CPP):
            res = _unpack_out(results[p][h]["out_packed"], rounds)
            out.reshape(G2, 32, B)[mask, :, h * NW:(h + 1) * NW] = res[mask]
    return out, np.float32(0.0)


# revision 17
# speedup vs baseline: 1.9132x; 1.0781x over previous
"""Bayesian block-sparse linear layer (gnn message passing) on 8 Trainium2 cores.

out = segment_sum_e( v[e].T @ x_block[col_g[e]] ) + bias,
v[e] = eps_w[e] * exp(weight_log_var[e]) + weight_mean[e]   (32x32 blocks)

Measured on TRN2: the PE is instruction-issue bound at ~33ns per
(LDWEIGHTS+MATMUL) pair regardless of the moving free dim N, and 32x32
sub-array matmuls (tile_position) run concurrently as long as no two
sub-arrays write the same (PSUM bank, partition range).  So the design
maximizes work per matmul instruction: N=512 batch columns per matmul.

Sharding: output blocks are split across 4 distinct specialized programs
(one per row-group of the graph), each run on 2 cores that hold the two
512-column batch halves.  Within a program:
  * all of x (one batch half) lives in SBUF, block c at partition group
    c%4, columns 512*(c//4);
  * per-edge weights are sampled on device (ACT exp + 2 DVE ops) from
    host-packed per-round layouts and consumed as matmul lhsT;
  * rounds of 8 output blocks; block (gg, s) accumulates in PSUM bank
    4*gg+g for PE row-group g (4 private copies -> concurrent sub-array
    matmuls never touch the same bank+partitions, which hard-crashes);
  * evacuation sums the 4 copies and adds the sampled bias.
"""

import os

import numpy as np

# problem dims (hardcoded per spec)
G1 = 256
G2 = 256
A1 = 32
A2 = 32
B = 1024
NCORES = 8

NPROG = 4
CPP = 2            # cores per program (batch halves)
NW = B // CPP      # 512 batch columns per core
NROUNDS = 8        # per program
BPR = 8            # blocks per round
BPP = G2 // NPROG  # blocks per program

LAST_PROFILE = None
_prog_cache = {}


def _dt_mode():
    return os.environ.get("BSL_DTYPE", "bf16")


# ---------------------------------------------------------------- host plan

def _plan(row_g, col_g):
    """Specialize schedules to the graph: 4 programs x 8 rounds x 8 blocks."""
    E = len(row_g)
    blk = [[[] for _ in range(4)] for _ in range(G2)]
    for e in range(E):
        blk[int(row_g[e])][int(col_g[e]) % 4].append(e)
    cnts = np.array([[len(blk[q][g]) for g in range(4)] for q in range(G2)])

    # blocks -> programs, balancing total edge count
    order = np.argsort(-cnts.sum(1), kind="stable")
    progs = [[] for _ in range(NPROG)]
    ptot = np.zeros(NPROG, np.int64)
    for q in order:
        cand = [p for p in range(NPROG) if len(progs[p]) < BPP]
        p = min(cand, key=lambda p: ptot[p])
        progs[p].append(int(q))
        ptot[p] += cnts[q].sum()

    plans = []
    for p in range(NPROG):
        # blocks -> rounds, balancing per-partition-group load
        rounds = [[] for _ in range(NROUNDS)]
        load = np.zeros((NROUNDS, 4), np.int64)
        for q in sorted(progs[p], key=lambda q: -cnts[q].sum()):
            best, bkey = None, None
            for r in range(NROUNDS):
                if len(rounds[r]) >= BPR:
                    continue
                nl = load[r] + cnts[q]
                key = (int(nl.max()), int(nl.sum()))
                if best is None or key < bkey:
                    best, bkey = r, key
            rounds[best].append(q)
            load[best] += cnts[q]

        blkmap = {}
        for r in range(NROUNDS):
            for idx, q in enumerate(rounds[r]):
                blkmap[q] = (r, idx)

        sched = []
        for r in range(NROUNDS):
            glists = [[] for _ in range(4)]
            for q in rounds[r]:
                for g in range(4):
                    lst = blk[q][g]
                    if lst:
                        for e in lst:
                            glists[g].append((e, q))
                    else:
                        # region never written otherwise -> dummy zero mm
                        glists[g].append((E, q))
            if r == 0:
                # round 0: order by x column so matmuls can start while the
                # chunked x DMA is still streaming in
                for g in range(4):
                    glists[g].sort(key=lambda eq: (
                        (int(col_g[eq[0]]) // 4) if eq[0] < E else 0))
            else:
                # bank-group-major, slot round-robin: consecutive same-group
                # matmuls hit different PE sub-arrays, and bank-group 0
                # finishes early so its PSUM evac overlaps group 1 matmuls
                for g in range(4):
                    byidx = [[] for _ in range(BPR)]
                    for e, q in glists[g]:
                        byidx[blkmap[q][1]].append((e, q))
                    inter = []
                    for ggv in range(BPR // 4):
                        active = byidx[4 * ggv:4 * ggv + 4]
                        i = 0
                        while any(active):
                            sl = active[i % 4]
                            if sl:
                                inter.append(sl.pop(0))
                            i += 1
                    glists[g] = inter
            L = max(len(x) for x in glists)
            q0 = rounds[r][0]
            for g in range(4):
                while len(glists[g]) < L:
                    glists[g].append((E, q0))
            # start/stop per (row-group g, block q): each (bank, slot)
            # region belongs to exactly one block copy, and only PE tile
            # (g, s) writes it -> the clear/accumulate order is the PE's
            # own FIFO, race-free.
            first_pos, last_pos = {}, {}
            for pp in range(L):
                for g in range(4):
                    e, q = glists[g][pp]
                    if (g, q) not in first_pos:
                        first_pos[(g, q)] = pp
                    last_pos[(g, q)] = pp
            entries = [[None] * L for _ in range(4)]
            widx = np.full((4, L), E, np.int64)
            for pp in range(L):
                for g in range(4):
                    e, q = glists[g][pp]
                    ridx = blkmap[q][1]
                    gg, s = ridx // 4, ridx % 4
                    xcol = (int(col_g[e]) // 4) if e < E else 0
                    entries[g][pp] = (gg, s, xcol,
                                      first_pos[(g, q)] == pp,
                                      last_pos[(g, q)] == pp)
                    widx[g, pp] = e
            sched.append({"L": L, "entries": entries, "widx": widx})
        plans.append({"sched": sched, "rounds": rounds})
    return plans


# ---------------------------------------------------------------- host pack

def _pack_weights(w, sched, np_dt):
    w = np.asarray(w, np.float32).reshape(-1, A1, A2)
    wext = np.concatenate([w, np.zeros((1, A1, A2), np.float32)], 0)
    outs = []
    for sc in sched:
        t = wext[sc["widx"]]                       # [4, L, 32, 32]
        t = t.transpose(0, 2, 1, 3).reshape(128, 32 * sc["L"])
        outs.append(np.ascontiguousarray(t.astype(np_dt)))
    return outs


def _pack_x(xk, np_dt):  # xk [8192, NW]
    t = xk.reshape(64, 4, 32, NW).transpose(1, 2, 0, 3)
    return np.ascontiguousarray(t.reshape(128, 64 * NW).astype(np_dt))


def _pack_bias(vec, rounds):  # vec [8192] fp32 -> [128, NROUNDS*2]
    out = np.zeros((NROUNDS, 2, 128), np.float32)
    for r in range(NROUNDS):
        for idx, q in enumerate(rounds[r]):
            gg, s = idx // 4, idx % 4
            out[r, gg, 32 * s:32 * s + 32] = vec[32 * q:32 * q + 32]
    return np.ascontiguousarray(out.transpose(2, 0, 1).reshape(128, NROUNDS * 2))


def _unpack_out(op, rounds):  # op [NROUNDS, 128, 2*NW] fp32 -> [BPP*32, NW]
    res = np.zeros((G2, 32, NW), np.float32)
    t = op.reshape(NROUNDS, 4, 32, 2, NW).transpose(0, 3, 1, 2, 4)
    for r in range(NROUNDS):
        k = t[r].reshape(8, 32, NW)          # idx = 4*gg + s
        for idx, q in enumerate(rounds[r]):
            res[q] = k[idx]
    return res


# ---------------------------------------------------------------- program

def _build(sched, dt_w, pidx):
    import concourse.bacc as bacc
    import concourse.mybir as mybir
    import concourse.tile as tile_mod

    nc = bacc.Bacc("TRN2", target_bir_lowering=False, debug=False,
                   num_devices=CPP)
    f32 = mybir.dt.float32
    AF = mybir.ActivationFunctionType
    ADD = mybir.AluOpType.add
    MUL = mybir.AluOpType.mult

    x_d = nc.dram_tensor("x_packed", [128, 64 * NW], dt_w,
                         kind="ExternalInput")
    wm_d = [nc.dram_tensor(f"wm_{r}", [128, 32 * sched[r]["L"]], dt_w,
                           kind="ExternalInput") for r in range(NROUNDS)]
    wl_d = [nc.dram_tensor(f"wl_{r}", [128, 32 * sched[r]["L"]], dt_w,
                           kind="ExternalInput") for r in range(NROUNDS)]
    we_d = [nc.dram_tensor(f"we_{r}", [128, 32 * sched[r]["L"]], dt_w,
                           kind="ExternalInput") for r in range(NROUNDS)]
    bm_d = nc.dram_tensor("bm_packed", [128, NROUNDS * 2], f32,
                          kind="ExternalInput")
    bl_d = nc.dram_tensor("bl_packed", [128, NROUNDS * 2], f32,
                          kind="ExternalInput")
    be_d = nc.dram_tensor("be_packed", [128, NROUNDS * 2], f32,
                          kind="ExternalInput")
    out_d = nc.dram_tensor("out_packed", [NROUNDS, 128, 2 * NW], f32,
                           kind="ExternalOutput")

    with tile_mod.TileContext(nc) as tc:
        with tc.tile_pool(name="xp", bufs=1) as xp, \
             tc.tile_pool(name="bp", bufs=1) as bp, \
             tc.tile_pool(name="wp", bufs=4) as wp, \
             tc.tile_pool(name="pp", bufs=8, space="PSUM") as pp, \
             tc.tile_pool(name="ep", bufs=4) as ep, \
             tc.tile_pool(name="opool", bufs=2) as opool:

            # x streams in column chunks; round-0 matmuls are sorted by
            # x column so they start as soon as their chunk lands
            xt = xp.tile([128, 64 * NW], dt_w, name="xt")
            XC = 16 * NW
            nc.sync.dma_start(xt[:, 0:XC], x_d.ap()[:, 0:XC])

            # round-0 weights right after the first x chunk, then the rest
            # of x, then biases: arrival order matches consumption order
            W0 = 32 * sched[0]["L"]
            wlt0 = wp.tile([128, W0], dt_w, tag="wl", name="wl_t0")
            nc.sync.dma_start(wlt0[:, :], wl_d[0].ap())
            wet0 = wp.tile([128, W0], dt_w, tag="we", name="we_t0")
            nc.sync.dma_start(wet0[:, :], we_d[0].ap())
            wmt0 = wp.tile([128, W0], dt_w, tag="wm", name="wm_t0")
            nc.sync.dma_start(wmt0[:, :], wm_d[0].ap())
            for c in range(1, 4):
                nc.sync.dma_start(xt[:, c * XC:(c + 1) * XC],
                                  x_d.ap()[:, c * XC:(c + 1) * XC])

            bmt = bp.tile([128, NROUNDS * 2], f32, name="bmt")
            nc.sync.dma_start(bmt[:, :], bm_d.ap())
            blt = bp.tile([128, NROUNDS * 2], f32, name="blt")
            nc.sync.dma_start(blt[:, :], bl_d.ap())
            bet = bp.tile([128, NROUNDS * 2], f32, name="bet")
            nc.sync.dma_start(bet[:, :], be_d.ap())
            bias = bp.tile([128, NROUNDS * 2], f32, name="bias")
            nc.scalar.activation(bias[:, :], blt[:, :], AF.Exp)
            nc.vector.tensor_tensor(bias[:, :], bias[:, :], bet[:, :], MUL)
            nc.vector.tensor_tensor(bias[:, :], bias[:, :], bmt[:, :], ADD)

            for r in range(NROUNDS):
                L = sched[r]["L"]
                W = 32 * L
                if r == 0:
                    wlt, wet, wmt = wlt0, wet0, wmt0
                else:
                    wlt = wp.tile([128, W], dt_w, tag="wl", name=f"wl_t{r}")
                    nc.sync.dma_start(wlt[:, :], wl_d[r].ap())
                    wet = wp.tile([128, W], dt_w, tag="we", name=f"we_t{r}")
                    nc.sync.dma_start(wet[:, :], we_d[r].ap())
                    wmt = wp.tile([128, W], dt_w, tag="wm", name=f"wm_t{r}")
                    nc.sync.dma_start(wmt[:, :], wm_d[r].ap())

                # v = eps * exp(log_var) + mean, in column halves so the
                # first matmuls can start before the full round is sampled
                half = 32 * ((L + 1) // 2)
                for c0, c1 in ((0, half), (half, W)):
                    if c0 >= c1:
                        continue
                    nc.scalar.activation(wlt[:, c0:c1], wlt[:, c0:c1], AF.Exp)
                    nc.vector.tensor_tensor(wet[:, c0:c1], wet[:, c0:c1],
                                            wlt[:, c0:c1], MUL)
                    nc.vector.tensor_tensor(wet[:, c0:c1], wet[:, c0:c1],
                                            wmt[:, c0:c1], ADD)

                # bank 4*gg+g is private to PE row-group g: no two
                # sub-array matmuls ever write the same (bank, partitions)
                banks = [pp.tile([128, NW], f32, tag="bank",
                                 name=f"bank{r}_{b}") for b in range(8)]
                ents = sched[r]["entries"]
                for pp_i in range(L):
                    for g in range(4):
                        gg, s, xcol, st, sp = ents[g][pp_i]
                        nc.tensor.matmul(
                            banks[4 * gg + g][32 * s:32 * s + 32, 0:NW],
                            lhsT=wet[32 * g:32 * g + 32,
                                     32 * pp_i:32 * pp_i + 32],
                            rhs=xt[32 * g:32 * g + 32,
                                   NW * xcol:NW * xcol + NW],
                            start=st, stop=sp, skip_group_check=True,
                            tile_position=(32 * g, 32 * s))

                # out = sum of the 4 row-group copies + bias
                # (DVE can take at most one PSUM operand per instruction)
                ot = opool.tile([128, 2 * NW], f32, tag="ot", name=f"ot{r}")
                for gg in range(2):
                    t1 = ep.tile([128, NW], f32, tag="eacc",
                                 name=f"t1_{r}_{gg}")
                    nc.scalar.activation(t1[:, :], banks[4 * gg][:, :],
                                         AF.Identity, bias=0.0)
                    for g in range(1, 4):
                        nc.vector.tensor_tensor(
                            t1[:, :], t1[:, :], banks[4 * gg + g][:, :], ADD)
                    nc.scalar.activation(
                        ot[:, NW * gg:NW * (gg + 1)], t1[:, :], AF.Identity,
                        bias=bias[:, 2 * r + gg:2 * r + gg + 1])
                nc.sync.dma_start(out_d.ap()[r, :, :], ot[:, :])

    nc.compile()
    return nc


# ---------------------------------------------------------------- runner

def _dispatch_pjrt(nc, in_maps, devices):
    """Async-dispatch one Bass program on a specific device subset.

    Clone of concourse.bass2jax.run_bass_via_pjrt's multi-core branch with
    an explicit device list; returns un-materialized jax arrays so several
    programs can execute concurrently on disjoint core pairs.
    """
    import jax
    import concourse.mybir as mybir
    from concourse import bass2jax
    from jax.sharding import Mesh, PartitionSpec
    from jax.experimental.shard_map import shard_map

    bass2jax.install_neuronx_cc_hook()
    assert nc.dbg_addr is None
    partition_name = (nc.partition_id_tensor.name
                      if nc.partition_id_tensor else None)

    in_names, out_names, out_avals, zero_outs = [], [], [], []
    for alloc in nc.m.functions[0].allocations:
        if not isinstance(alloc, mybir.MemoryLocationSet):
            continue
        name = alloc.memorylocations[0].name
        if alloc.kind == "ExternalInput":
            if name != partition_name:
                in_names.append(name)
        elif alloc.kind == "ExternalOutput":
            out_names.append(name)
            shape = tuple(alloc.tensor_shape)
            dtype = mybir.dt.np(alloc.dtype)
            out_avals.append(jax.core.ShapedArray(shape, dtype))
            zero_outs.append(np.zeros(shape, dtype))
    n_params = len(in_names)
    n_outs = len(out_avals)
    in_names = in_names + out_names
    if partition_name is not None:
        in_names.append(partition_name)
    donate = tuple(range(n_params, n_params + n_outs))

    def _body(*args):
        operands = list(args)
        if partition_name is not None:
            operands.append(bass2jax.partition_id_tensor())
        outs = bass2jax._bass_exec_p.bind(
            *operands,
            out_avals=tuple(out_avals),
            in_names=tuple(in_names),
            out_names=tuple(out_names),
            lowering_input_output_aliases=(),
            sim_require_finite=True,
            sim_require_nnan=True,
            nc=nc,
        )
        return tuple(outs)

    n_cores = len(devices)
    mesh = Mesh(np.asarray(devices), ("core",))
    sharded = jax.jit(
        shard_map(_body, mesh=mesh,
                  in_specs=(PartitionSpec("core"),) * (n_params + n_outs),
                  out_specs=(PartitionSpec("core"),) * n_outs,
                  check_rep=False),
        donate_argnums=donate, keep_unused=True)
    per_core = [[np.asarray(m[name]) for name in in_names[:n_params]]
                for m in in_maps]
    concat_in = [np.concatenate([per_core[c][i] for c in range(n_cores)], 0)
                 for i in range(n_params)]
    concat_zeros = [np.zeros((n_cores * z.shape[0], *z.shape[1:]), z.dtype)
                    for z in zero_outs]
    out_arrs = sharded(*concat_in, *concat_zeros)
    return out_arrs, out_names, out_avals, n_cores


def _run_programs(ncs, maps_list, trace):
    """Run the programs concurrently on disjoint device pairs."""
    import jax
    devices = jax.devices()

    def dispatch_all():
        handles = []
        for q, (nc, maps) in enumerate(zip(ncs, maps_list)):
            devs = devices[CPP * q:CPP * (q + 1)]
            handles.append(_dispatch_pjrt(nc, maps, devs))
        return handles

    prof = {"exec_time_ns": None, "mean_exec_time_ns": None,
            "max_exec_time_core_id": None, "trace": None, "insts": None,
            "per_core_ns": None}

    if not trace:
        handles = dispatch_all()
    else:
        handles = None
        try:
            import glob as globmod
            import re
            import shutil
            import tempfile
            import time as time_mod
            from antenv.axon_hooks import get_axon_ntff_profile_hook
            hook = get_axon_ntff_profile_hook()
            neff_dir = tempfile.mkdtemp()
            with hook(neff_dir, list(range(NCORES))):
                t0 = time_mod.time()
                handles = dispatch_all()
                for out_arrs, _, _, _ in handles:
                    for a in out_arrs:
                        a.block_until_ready()
                wall_s = time_mod.time() - t0
            prof["wall_s"] = wall_s
            ntffs = globmod.glob(os.path.join(neff_dir, "*_body*.ntff"))
            if ntffs:
                import gauge.profiler
                from concourse._compat import FishPath
                # group by executable (one per program, in dispatch order)
                exids = sorted({re.search(r"executable(\d+)", f).group(1)
                                for f in ntffs})
                per_core = {}
                insts = None
                tracep = None
                best = -1
                for qi, exid in enumerate(exids):
                    sub = os.path.join(neff_dir, f"ex{exid}")
                    os.makedirs(sub, exist_ok=True)
                    for f in os.listdir(neff_dir):
                        if f"executable{exid}" in f:
                            shutil.move(os.path.join(neff_dir, f),
                                        os.path.join(sub, f))
                    profile = gauge.profiler.Profile(
                        profile_path=FishPath(sub),
                        kernel_dev_mode=True, profile_on_exit=False,
                        bass_kernel=ncs[min(qi, len(ncs) - 1)].m,
                        offline_processing=True, fname="*_body*")
                    results = profile.to_perfetto(model_index=(0, 1))
                    for ci, pr in enumerate(results or []):
                        per_core[(qi, ci)] = pr.exec_time_ns
                        if (pr.exec_time_ns or 0) > best:
                            best = pr.exec_time_ns or 0
                            insts, tracep = pr.insts, pr.trace_path
                vals = [v for v in per_core.values() if v]
                if vals:
                    prof.update(
                        exec_time_ns=max(vals),
                        mean_exec_time_ns=sum(vals) / len(vals),
                        max_exec_time_core_id=str(max(
                            per_core, key=lambda c: per_core[c] or 0)),
                        per_core_ns={str(k): v for k, v in per_core.items()},
                        insts=insts, trace=tracep)
        except Exception as exc:  # profiling must never break the run
            print(f"[kernel] trace failed: {type(exc).__name__}: {exc}")
            if handles is None:
                handles = dispatch_all()

    results = []
    for out_arrs, out_names, out_avals, n_cores in handles:
        cores = []
        for c in range(n_cores):
            cores.append({
                name: np.asarray(out_arrs[i]).reshape(
                    n_cores, *out_avals[i].shape)[c]
                for i, name in enumerate(out_names)})
        results.append(cores)
    return results, prof


# ---------------------------------------------------------------- entry

def _get_programs(row_g, col_g):
    import concourse.mybir as mybir
    import ml_dtypes
    mode = _dt_mode()
    key = (row_g.tobytes(), col_g.tobytes(), mode)
    if key not in _prog_cache:
        plans = _plan(row_g, col_g)
        dt_w = mybir.dt.float32 if mode == "fp32" else mybir.dt.bfloat16
        np_dt = np.float32 if mode == "fp32" else ml_dtypes.bfloat16
        ncs = [_build(plans[p]["sched"], dt_w, p) for p in range(NPROG)]
        _prog_cache[key] = (ncs, plans, np_dt)
    return _prog_cache[key]


def kernel(**inputs):
    global LAST_PROFILE

    row_g = np.asarray(inputs["row_g"])
    col_g = np.asarray(inputs["col_g"])
    ncs, plans, np_dt = _get_programs(row_g, col_g)

    x = np.asarray(inputs["x"], np.float32)
    xpk = [_pack_x(np.ascontiguousarray(x[:, h * NW:(h + 1) * NW]), np_dt)
           for h in range(CPP)]
    bm = np.asarray(inputs["b_mean"], np.float32)
    bl = np.asarray(inputs["b_log_var"], np.float32)
    be = np.asarray(inputs["eps_b"], np.float32)

    maps_list = []
    for p in range(NPROG):
        sched, rounds = plans[p]["sched"], plans[p]["rounds"]
        wm = _pack_weights(inputs["weight_mean"], sched, np_dt)
        wl = _pack_weights(inputs["weight_log_var"], sched, np_dt)
        we = _pack_weights(inputs["eps_w"], sched, np_dt)
        shared = {f"wm_{r}": wm[r] for r in range(NROUNDS)}
        shared.update({f"wl_{r}": wl[r] for r in range(NROUNDS)})
        shared.update({f"we_{r}": we[r] for r in range(NROUNDS)})
        shared["bm_packed"] = _pack_bias(bm, rounds)
        shared["bl_packed"] = _pack_bias(bl, rounds)
        shared["be_packed"] = _pack_bias(be, rounds)
        maps_list.append([{**shared, "x_packed": xpk[h]} for h in range(CPP)])

    trace = os.environ.get("BSL_TRACE", "0") == "1"
    results, prof = _run_programs(ncs, maps_list, trace)
    LAST_PROFILE = prof

    out = np.zeros((G2 * A2, B), np.float32)
    for p in range(NPROG):
        rounds = plans[p]["rounds"]
        mask = np.zeros(G2, bool)
        for r in range(NROUNDS):
            for q in rounds[r]:
                mask[q] = True
        rows = np.repeat(mask, 32)
        for h in range(CPP):
            res = _unpack_out(results[p][h]["out_packed"], rounds)
            out.reshape(G2, 32, B)[mask, :, h * NW:(h + 1) * NW] = res[mask]
    return out, np.float32(0.0)
